# revision 29
# baseline (speedup 1.0000x reference)
"""Contrastive (Cauchy-kernel InfoNCE) loss on 8 Trainium2 NeuronCores.

Math: for anchors a_i = features[i] (i < b) and the canonical full-batch
neighbor indices (pos = i+b in column 0, negatives = everything except
self in both halves), the loss reduces to

    D[i, n]  = 1 + ||a_i||^2 + ||f_n||^2 - 2 a_i . f_n
    P[i, n]  = 1 / D[i, n]                              (Cauchy probit)
    S_i      = sum_n P[i, n] - 1                        (P[i,i] == 1 exactly)
    loss     = mean_i [ ln(S_i) + ln(D[i, i+b]) ]

With r_i = 1/(1 + ||a_i||^2), the ACT engine computes in ONE pass
    Q[i, n] = 1/(D''[i, n] * r_i + 1)  =  (1 + ||a_i||^2) * P[i, n]
(via the activation's per-partition scale operand, immediate bias 1.0),
where D''[i, n] = ||f_n||^2 - 2 a_i . f_n comes from two matmuls. Row sums
of Q then give S_i = r_i * sum_n Q[i, n].

Sharding: data-parallel over anchors. Core c owns anchors c*128..(c+1)*128.
Each core receives the full feature matrix in bf16, pre-transposed
([dim, 2b]) and block-permuted so its own anchor block is columns 0:128
and its positive block is columns 128:256. Each core emits the partial
loss sum of its 128 anchors; the host sums 8 scalars and divides by b.
"""

import numpy as np
import orjson

import concourse.bass as bass
import concourse.bass2jax as bass2jax
import concourse.bass_utils as bass_utils
import concourse.mybir as mybir
import concourse.tile as tile
from concourse.masks import make_identity
from concourse.bass_utils import run_bass_kernel_spmd

B = 1024
DIM = 128
N = 2 * B            # 2048 feature rows
NCORES = 8
PB = B // NCORES     # 128 anchors per core
CH = 512             # DMA chunk columns
HALF = 1024          # ACT probit pass width (spans 2 PSUM banks)
F32 = mybir.dt.float32
BF16 = mybir.dt.bfloat16
FP8 = mybir.dt.float8e4   # e4m3

NJUNK = 3            # wide PE warmup matmuls (HAM clock ramp)

# Set by a driver to profile the HW execution (requires an NTFF hook).
TRACE = False
LAST_RESULT = None


def _split_multi_waits(bir_json: bytes) -> bytes:
    """The walrus build here accepts only ONE sync-wait per instruction,
    while Tile freely attaches several (one per producer proc). Engines pop
    their queues in order, so hoisting the extra waits onto injected NoOps
    immediately before the instruction is semantically identical."""
    m = orjson.loads(bir_json)
    changed = False
    for fn in m.get("functions", []):
        for blk in fn.get("blocks", []):
            out = []
            for inst in blk.get("instructions", []):
                si = inst.get("sync_info")
                ow = (si or {}).get("on_wait") or []
                if len(ow) > 1:
                    changed = True
                    for k, w in enumerate(ow[:-1]):
                        out.append(
                            {
                                "debug": inst.get("debug", 0),
                                "engine": inst["engine"],
                                "ins": [],
                                "outs": [],
                                "name": f"{inst['name']}-w{k}",
                                "opcode": "NoOp",
                                "text_hint": "wait_split",
                                "sync_info": {"on_update": [], "on_wait": [w]},
                            }
                        )
                    si["on_wait"] = [ow[-1]]
                if inst.get("op_name") == "EVENT_SEMAPHORE_RANGE_CLEAR":
                    inst["engine"] = "SP"
                    changed = True
                out.append(inst)
            blk["instructions"] = out
    return orjson.dumps(m) if changed else bir_json


def _patch_compiler():
    if getattr(bass_utils, "_wait_split_patch", False):
        return
    orig = bass_utils.compile_bir_kernel

    def patched(bir_json, tmpdir, neff_name="file.neff"):
        return orig(_split_multi_waits(bir_json), tmpdir, neff_name=neff_name)

    bass_utils.compile_bir_kernel = patched
    bass2jax.compile_bir_kernel = patched
    bass_utils._wait_split_patch = True


def _act_recip(nc, out, in_, scale, bias=1.0, accum_out=None):
    """ACT Reciprocal activation: out = 1/(in_*scale + bias).

    bass.activation() refuses Reciprocal outright (it has table-grade
    accuracy), but this loss only needs ~1e-3 on a 2047-term average, so
    emit the InstActivation directly. bias must be an immediate here
    (walrus sundagen requirement for Copy/Reciprocal); scale may be a
    per-partition [128,1] AP."""
    eng = nc.scalar
    inputs = [eng.lower_ap(in_)]
    for arg in (float(bias), scale, 0.0):
        if isinstance(arg, float):
            inputs.append(mybir.ImmediateValue(dtype=mybir.dt.float32, value=arg))
        else:
            inputs.append(eng.lower_ap(arg))
    outputs = [eng.lower_ap(out)]
    if accum_out is not None:
        outputs.append(eng.lower_ap(accum_out))
    return eng.add_instruction(
        mybir.InstActivation(
            name=nc.get_next_instruction_name(),
            func=mybir.ActivationFunctionType.Reciprocal,
            ins=inputs,
            outs=outputs,
        )
    )


def _build_canonical():
    """Per-core program: ftp [DIM, N] (bf16, transposed, block-permuted
    features) -> out [1, 1] partial loss sum over this core's 128 anchors."""
    _patch_compiler()
    nc = bass.Bass(enable_partition_id=False)
    ftp = nc.dram_tensor("ftp", [DIM, N], BF16, kind="ExternalInput")
    out = nc.dram_tensor("out", [1, 1], F32, kind="ExternalOutput")

    with tile.TileContext(nc) as tc:
        with (
            tc.tile_pool(name="sb", bufs=1) as sb,
            tc.tile_pool(name="psum", bufs=1, space="PSUM") as psum,
        ):
            big = small = consts = sb
            # ---- constants ----
            warm_in = small.tile([1, 1], F32, tag="warm_in")
            nc.vector.memset(warm_in, 1.0)
            ones128 = consts.tile([128, 128], BF16, tag="ones128")
            nc.vector.memset(ones128, 1.0)
            onec_bf = consts.tile([128, 1], BF16, tag="onec_bf")
            nc.vector.memset(onec_bf, 1.0)
            ident = consts.tile([128, 128], BF16, tag="ident")
            make_identity(nc, ident)
            warm_rhs = big.tile([128, 128], BF16, tag="warm_rhs")
            nc.gpsimd.memset(warm_rhs, 0.0)

            # Warm the ACT reciprocal table early so the ~1.5us table load
            # overlaps with DMA instead of gating the first probit pass.
            recwarm = small.tile([1, 1], F32, tag="recwarm")
            _act_recip(nc, recwarm, warm_in, 1.0)

            ft = big.tile([128, N], BF16, tag="ft")
            ft2 = big.tile([128, N], BF16, tag="ft2")

            # features stream in; each 512-col chunk is split across BOTH
            # DMA queues so chunk 0 (and with it the whole compute pipeline)
            # completes as early as possible
            for j in range(4):
                lo, mid, hi = j * CH, j * CH + CH // 2, (j + 1) * CH
                nc.sync.dma_start(out=ft[:, lo:mid], in_=ftp[:, lo:mid])
                nc.scalar.dma_start(out=ft[:, mid:hi], in_=ftp[:, mid:hi])

            # Wide dummy matmuls while DMA streams in: sustained PE activity
            # ramps the HAM utilization limit before the real matmuls.
            junk = psum.tile([128, 512], F32, tag="junk")
            warm_rhs512 = big.tile([128, 512], BF16, tag="warm_rhs512")
            nc.gpsimd.memset(warm_rhs512, 0.0)
            for _ in range(NJUNK):
                nc.tensor.matmul(junk, ones128, warm_rhs512, start=True, stop=True)

            # ---- squares (bf16, DVE) + D'' matmuls, pipelined per chunk ----
            bankA = psum.tile([128, HALF], F32, tag="bankA")
            bankB = psum.tile([128, HALF], F32, tag="bankB")
            banks = [bankA, bankA, bankB, bankB]
            pcol = psum.tile([128, 1], F32, tag="pcol")

            atm2 = small.tile([128, 128], BF16, tag="atm2")
            nc.vector.tensor_scalar_mul(atm2, ft[:, 0:128], -2.0)
            nc.vector.tensor_mul(ft2[:, 0:CH], ft[:, 0:CH], ft[:, 0:CH])
            nc.vector.tensor_mul(ft2[:, CH:2 * CH], ft[:, CH:2 * CH], ft[:, CH:2 * CH])

            # PE: gram c0, then the tiny anchor-norm matmul, then the rest
            nc.tensor.matmul(bankA[:, 0:CH], atm2, ft[:, 0:CH], start=True, stop=False)
            nc.tensor.matmul(pcol, ft2[:, 0:128], onec_bf, start=True, stop=True)
            nc.tensor.matmul(bankA[:, 0:CH], ones128, ft2[:, 0:CH], start=False, stop=True)

            # asq1 = 1 + ||a||^2 and its reciprocal (probit scale operand)
            asq1_col = small.tile([128, 1], F32, tag="asq1_col")
            nc.vector.tensor_scalar_add(asq1_col, pcol, 1.0)
            r_col = small.tile([128, 1], F32, tag="r_col")
            nc.vector.reciprocal(out=r_col, in_=asq1_col)

            nc.vector.tensor_mul(ft2[:, 2 * CH:3 * CH], ft[:, 2 * CH:3 * CH], ft[:, 2 * CH:3 * CH])
            nc.vector.tensor_mul(ft2[:, 3 * CH:N], ft[:, 3 * CH:N], ft[:, 3 * CH:N])

            for j in range(1, 4):
                bank = banks[j]
                bsl = slice((j % 2) * CH, (j % 2) * CH + CH)
                fsl = slice(j * CH, (j + 1) * CH)
                nc.tensor.matmul(bank[:, bsl], atm2, ft[:, fsl], start=True, stop=False)
                nc.tensor.matmul(
                    bank[:, bsl], ones128, ft2[:, fsl], start=False, stop=True
                )

            # ---- probits: Q = 1/(D''*r + 1); S = r * rowsum(Q) - 1 ----
            # (Q[i,i] = 1/(1 + 2*eps*r) with eps the bf16 squares' rounding
            # error; |2*eps| < 0.4 so this stays well-conditioned and the
            # residual after subtracting 1 averages out over anchors.)
            sparts = small.tile([128, 2], F32, tag="sparts")
            probA = big.tile([128, HALF], BF16, tag="probA")
            probB = big.tile([128, HALF], BF16, tag="probB")
            _act_recip(nc, probA, bankA, r_col, 1.0, accum_out=sparts[:, 0:1])
            _act_recip(nc, probB, bankB, r_col, 1.0, accum_out=sparts[:, 1:2])

            # ---- positive distances: diag of bankA cols 128:256 ----
            scrp = small.tile([128, 128], BF16, tag="scrp")
            nc.vector.tensor_mul(scrp, bankA[:, 128:256], ident)
            posd0 = small.tile([128, 1], F32, tag="posd0")
            nc.vector.tensor_reduce(
                posd0, scrp, axis=mybir.AxisListType.X, op=mybir.AluOpType.add
            )
            posd = small.tile([128, 1], F32, tag="posd")
            nc.vector.tensor_tensor(posd, posd0, asq1_col, mybir.AluOpType.add)

            # ---- tail: loss_i = ln((r*S'_i - 1) * D_pos_i), partial sum ----
            s_all = small.tile([128, 1], F32, tag="s_all")
            nc.vector.tensor_reduce(
                s_all, sparts, axis=mybir.AxisListType.X, op=mybir.AluOpType.add
            )
            snet = small.tile([128, 1], F32, tag="snet")
            nc.vector.tensor_scalar(
                snet, s_all, r_col, 1.0, mybir.AluOpType.mult,
                mybir.AluOpType.subtract,
            )
            tailv = small.tile([128, 1], F32, tag="tailv")
            nc.vector.tensor_tensor(tailv, snet, posd, mybir.AluOpType.mult)
            ltail = small.tile([128, 1], F32, tag="ltail")
            nc.scalar.activation(ltail, tailv, mybir.ActivationFunctionType.Ln)
            # partition-sum on GpSimd (cross-lane add) replaces the ones
            # matmul + PSUM copy of earlier revisions
            lsum = small.tile([1, 1], F32, tag="lsum")
            nc.gpsimd.tensor_reduce(
                lsum, ltail, axis=mybir.AxisListType.C, op=mybir.AluOpType.add
            )
            nc.sync.dma_start(out=out[:, :], in_=lsum)

    return nc


_NC = None


def _canonical_inds():
    idx = np.arange(B)
    not_self = ~np.eye(B, dtype=bool)
    neg1 = np.broadcast_to(idx[None, :], (B, B))[not_self].reshape(B, B - 1)
    neg2 = neg1 + B
    pos = (idx + B)[:, None]
    return np.concatenate([pos, neg1, neg2], axis=1)


_CANON = None


def _is_canonical(neigh_inds):
    global _CANON
    if neigh_inds.shape != (B, 2 * B - 1):
        return False
    if _CANON is None:
        _CANON = _canonical_inds()
    return np.array_equal(np.asarray(neigh_inds, dtype=np.int64), _CANON)


def _run_fast(feats):
    global _NC, LAST_RESULT

    if _NC is None:
        _NC = _build_canonical()
    fb = feats.astype(mybir.dt.np(BF16))
    in_maps = []
    for c in range(NCORES):
        order = [c, NCORES + c] + [
            blk for blk in range(16) if blk not in (c, NCORES + c)
        ]
        rows = np.concatenate([np.arange(blk * 128, (blk + 1) * 128) for blk in order])
        ftp = np.ascontiguousarray(fb[rows].T)
        in_maps.append({"ftp": ftp})
    res = run_bass_kernel_spmd(_NC, in_maps, list(range(NCORES)), trace=TRACE)
    LAST_RESULT = res
    total = sum(float(res.results[c]["out"][0, 0]) for c in range(NCORES))
    return np.asarray(total / B, dtype=np.float32)


def _run_general(feats, neigh_inds):
    """Correctness fallback for non-canonical neighbor indices."""
    b = feats.shape[0] // 2
    origs = feats[:b]
    gram = origs @ feats.T
    sq = np.sum(feats * feats, axis=1)
    dists = sq[:b, None] + sq[None, :] - 2.0 * gram
    probs = 1.0 / (1.0 + dists)
    rows = np.arange(b)[:, None]
    sel = probs[rows, np.asarray(neigh_inds, dtype=np.int64)]
    loss = -(np.log(sel[:, 0]) - np.log(np.sum(sel, axis=1)))
    return np.asarray(np.mean(loss), dtype=np.float32)


def kernel(features, neigh_inds):
    feats = np.ascontiguousarray(np.asarray(features, dtype=np.float32))
    ni = np.asarray(neigh_inds)
    if _is_canonical(ni):
        return _run_fast(feats)
    return _run_general(feats, ni)


# revision 31
# speedup vs baseline: 1.0086x; 1.0086x over previous
"""Contrastive (Cauchy-kernel InfoNCE) loss on 8 Trainium2 NeuronCores.

Math: for anchors a_i = features[i] (i < b) and the canonical full-batch
neighbor indices (pos = i+b in column 0, negatives = everything except
self in both halves), the loss reduces to

    D[i, n]  = 1 + ||a_i||^2 + ||f_n||^2 - 2 a_i . f_n
    P[i, n]  = 1 / D[i, n]                              (Cauchy probit)
    S_i      = sum_n P[i, n] - 1                        (P[i,i] == 1 exactly)
    loss     = mean_i [ ln(S_i) + ln(D[i, i+b]) ]

With r_i = 1/(1 + ||a_i||^2), the ACT engine computes in ONE pass
    Q[i, n] = 1/(D''[i, n] * r_i + 1)  =  (1 + ||a_i||^2) * P[i, n]
(via the activation's per-partition scale operand, immediate bias 1.0),
where D''[i, n] = ||f_n||^2 - 2 a_i . f_n comes from two matmuls. Row sums
of Q then give S_i = r_i * sum_n Q[i, n].

Sharding: data-parallel over anchors. Core c owns anchors c*128..(c+1)*128.
Each core receives the full feature matrix in bf16, pre-transposed
([dim, 2b]) and block-permuted so its own anchor block is columns 0:128
and its positive block is columns 128:256. Each core emits the partial
loss sum of its 128 anchors; the host sums 8 scalars and divides by b.
"""

import numpy as np
import orjson

import concourse.bass as bass
import concourse.bass2jax as bass2jax
import concourse.bass_utils as bass_utils
import concourse.mybir as mybir
import concourse.tile as tile
from concourse.masks import make_identity
from concourse.bass_utils import run_bass_kernel_spmd

B = 1024
DIM = 128
N = 2 * B            # 2048 feature rows
NCORES = 8
PB = B // NCORES     # 128 anchors per core
CH = 512             # DMA chunk columns
HALF = 1024          # ACT probit pass width (spans 2 PSUM banks)
F32 = mybir.dt.float32
BF16 = mybir.dt.bfloat16
FP8 = mybir.dt.float8e4   # e4m3

NJUNK = 4            # wide PE warmup matmuls (HAM clock ramp)

# Set by a driver to profile the HW execution (requires an NTFF hook).
TRACE = False
LAST_RESULT = None


def _split_multi_waits(bir_json: bytes) -> bytes:
    """The walrus build here accepts only ONE sync-wait per instruction,
    while Tile freely attaches several (one per producer proc). Engines pop
    their queues in order, so hoisting the extra waits onto injected NoOps
    immediately before the instruction is semantically identical."""
    m = orjson.loads(bir_json)
    changed = False
    for fn in m.get("functions", []):
        for blk in fn.get("blocks", []):
            out = []
            for inst in blk.get("instructions", []):
                si = inst.get("sync_info")
                ow = (si or {}).get("on_wait") or []
                if len(ow) > 1:
                    changed = True
                    for k, w in enumerate(ow[:-1]):
                        out.append(
                            {
                                "debug": inst.get("debug", 0),
                                "engine": inst["engine"],
                                "ins": [],
                                "outs": [],
                                "name": f"{inst['name']}-w{k}",
                                "opcode": "NoOp",
                                "text_hint": "wait_split",
                                "sync_info": {"on_update": [], "on_wait": [w]},
                            }
                        )
                    si["on_wait"] = [ow[-1]]
                if inst.get("op_name") == "EVENT_SEMAPHORE_RANGE_CLEAR":
                    inst["engine"] = "SP"
                    changed = True
                out.append(inst)
            blk["instructions"] = out
    return orjson.dumps(m) if changed else bir_json


def _patch_compiler():
    if getattr(bass_utils, "_wait_split_patch", False):
        return
    orig = bass_utils.compile_bir_kernel

    def patched(bir_json, tmpdir, neff_name="file.neff"):
        return orig(_split_multi_waits(bir_json), tmpdir, neff_name=neff_name)

    bass_utils.compile_bir_kernel = patched
    bass2jax.compile_bir_kernel = patched
    bass_utils._wait_split_patch = True


def _act_recip(nc, out, in_, scale, bias=1.0, accum_out=None):
    """ACT Reciprocal activation: out = 1/(in_*scale + bias).

    bass.activation() refuses Reciprocal outright (it has table-grade
    accuracy), but this loss only needs ~1e-3 on a 2047-term average, so
    emit the InstActivation directly. bias must be an immediate here
    (walrus sundagen requirement for Copy/Reciprocal); scale may be a
    per-partition [128,1] AP."""
    eng = nc.scalar
    inputs = [eng.lower_ap(in_)]
    for arg in (float(bias), scale, 0.0):
        if isinstance(arg, float):
            inputs.append(mybir.ImmediateValue(dtype=mybir.dt.float32, value=arg))
        else:
            inputs.append(eng.lower_ap(arg))
    outputs = [eng.lower_ap(out)]
    if accum_out is not None:
        outputs.append(eng.lower_ap(accum_out))
    return eng.add_instruction(
        mybir.InstActivation(
            name=nc.get_next_instruction_name(),
            func=mybir.ActivationFunctionType.Reciprocal,
            ins=inputs,
            outs=outputs,
        )
    )


def _build_canonical():
    """Per-core program: ftp [DIM, N] (bf16, transposed, block-permuted
    features) -> out [1, 1] partial loss sum over this core's 128 anchors."""
    _patch_compiler()
    nc = bass.Bass(enable_partition_id=False)
    ftp = nc.dram_tensor("ftp", [DIM, N], BF16, kind="ExternalInput")
    out = nc.dram_tensor("out", [1, 1], F32, kind="ExternalOutput")

    with tile.TileContext(nc) as tc:
        with (
            tc.tile_pool(name="sb", bufs=1) as sb,
            tc.tile_pool(name="psum", bufs=1, space="PSUM") as psum,
        ):
            big = small = consts = sb
            # ---- constants ----
            warm_in = small.tile([1, 1], F32, tag="warm_in")
            nc.vector.memset(warm_in, 1.0)
            ones128 = consts.tile([128, 128], BF16, tag="ones128")
            nc.vector.memset(ones128, 1.0)
            onec_bf = consts.tile([128, 1], BF16, tag="onec_bf")
            nc.vector.memset(onec_bf, 1.0)
            ident = consts.tile([128, 128], BF16, tag="ident")
            make_identity(nc, ident)
            warm_rhs = big.tile([128, 128], BF16, tag="warm_rhs")
            nc.gpsimd.memset(warm_rhs, 0.0)

            # Warm the ACT reciprocal table early so the ~1.5us table load
            # overlaps with DMA instead of gating the first probit pass.
            recwarm = small.tile([1, 1], F32, tag="recwarm")
            _act_recip(nc, recwarm, warm_in, 1.0)

            ft = big.tile([128, N], BF16, tag="ft")
            ft2 = big.tile([128, N], BF16, tag="ft2")

            # features stream in; chunk 0 is split across BOTH DMA queues so
            # the whole compute pipeline starts as early as possible
            nc.sync.dma_start(out=ft[:, 0:CH // 2], in_=ftp[:, 0:CH // 2])
            nc.scalar.dma_start(out=ft[:, CH // 2:CH], in_=ftp[:, CH // 2:CH])
            nc.scalar.dma_start(out=ft[:, CH:2 * CH], in_=ftp[:, CH:2 * CH])
            nc.sync.dma_start(out=ft[:, 2 * CH:3 * CH], in_=ftp[:, 2 * CH:3 * CH])
            nc.scalar.dma_start(out=ft[:, 3 * CH:N], in_=ftp[:, 3 * CH:N])

            # Wide dummy matmuls while DMA streams in: sustained PE activity
            # ramps the HAM utilization limit before the real matmuls.
            junk = psum.tile([128, 512], F32, tag="junk")
            warm_rhs512 = big.tile([128, 512], BF16, tag="warm_rhs512")
            nc.gpsimd.memset(warm_rhs512, 0.0)
            for _ in range(NJUNK):
                nc.tensor.matmul(junk, ones128, warm_rhs512, start=True, stop=True)

            # ---- squares (bf16, DVE) + D'' matmuls, pipelined per chunk ----
            bankA = psum.tile([128, HALF], F32, tag="bankA")
            bankB = psum.tile([128, HALF], F32, tag="bankB")
            banks = [bankA, bankA, bankB, bankB]
            pcol = psum.tile([128, 1], F32, tag="pcol")

            atm2 = small.tile([128, 128], BF16, tag="atm2")
            nc.vector.tensor_scalar_mul(atm2, ft[:, 0:128], -2.0)
            nc.vector.tensor_mul(ft2[:, 0:CH], ft[:, 0:CH], ft[:, 0:CH])
            nc.vector.tensor_mul(ft2[:, CH:2 * CH], ft[:, CH:2 * CH], ft[:, CH:2 * CH])

            # PE: gram c0, then the tiny anchor-norm matmul, then the rest
            nc.tensor.matmul(bankA[:, 0:CH], atm2, ft[:, 0:CH], start=True, stop=False)
            nc.tensor.matmul(pcol, ft2[:, 0:128], onec_bf, start=True, stop=True)
            nc.tensor.matmul(bankA[:, 0:CH], ones128, ft2[:, 0:CH], start=False, stop=True)

            # asq1 = 1 + ||a||^2 and its reciprocal (probit scale operand)
            asq1_col = small.tile([128, 1], F32, tag="asq1_col")
            nc.vector.tensor_scalar_add(asq1_col, pcol, 1.0)
            r_col = small.tile([128, 1], F32, tag="r_col")
            nc.vector.reciprocal(out=r_col, in_=asq1_col)

            nc.vector.tensor_mul(ft2[:, 2 * CH:3 * CH], ft[:, 2 * CH:3 * CH], ft[:, 2 * CH:3 * CH])
            nc.vector.tensor_mul(ft2[:, 3 * CH:N], ft[:, 3 * CH:N], ft[:, 3 * CH:N])

            for j in range(1, 4):
                bank = banks[j]
                bsl = slice((j % 2) * CH, (j % 2) * CH + CH)
                fsl = slice(j * CH, (j + 1) * CH)
                nc.tensor.matmul(bank[:, bsl], atm2, ft[:, fsl], start=True, stop=False)
                nc.tensor.matmul(
                    bank[:, bsl], ones128, ft2[:, fsl], start=False, stop=True
                )

            # ---- probits: Q = 1/(D''*r + 1); S = r * rowsum(Q) - 1 ----
            # (Q[i,i] = 1/(1 + 2*eps*r) with eps the bf16 squares' rounding
            # error; |2*eps| < 0.4 so this stays well-conditioned and the
            # residual after subtracting 1 averages out over anchors.)
            sparts = small.tile([128, 2], F32, tag="sparts")
            probA = big.tile([128, HALF], BF16, tag="probA")
            probB = big.tile([128, HALF], BF16, tag="probB")
            _act_recip(nc, probA, bankA, r_col, 1.0, accum_out=sparts[:, 0:1])
            _act_recip(nc, probB, bankB, r_col, 1.0, accum_out=sparts[:, 1:2])

            # ---- positive distances: diag of bankA cols 128:256 ----
            scrp = small.tile([128, 128], BF16, tag="scrp")
            nc.vector.tensor_mul(scrp, bankA[:, 128:256], ident)
            posd0 = small.tile([128, 1], F32, tag="posd0")
            nc.vector.tensor_reduce(
                posd0, scrp, axis=mybir.AxisListType.X, op=mybir.AluOpType.add
            )
            posd = small.tile([128, 1], F32, tag="posd")
            nc.vector.tensor_tensor(posd, posd0, asq1_col, mybir.AluOpType.add)

            # ---- tail: loss_i = ln((r*S'_i - 1) * D_pos_i), partial sum ----
            s_all = small.tile([128, 1], F32, tag="s_all")
            nc.vector.tensor_reduce(
                s_all, sparts, axis=mybir.AxisListType.X, op=mybir.AluOpType.add
            )
            snet = small.tile([128, 1], F32, tag="snet")
            nc.vector.tensor_scalar(
                snet, s_all, r_col, 1.0, mybir.AluOpType.mult,
                mybir.AluOpType.subtract,
            )
            tailv = small.tile([128, 1], F32, tag="tailv")
            nc.vector.tensor_tensor(tailv, snet, posd, mybir.AluOpType.mult)
            ltail = small.tile([128, 1], F32, tag="ltail")
            nc.scalar.activation(ltail, tailv, mybir.ActivationFunctionType.Ln)
            # partition-sum on GpSimd (cross-lane add) replaces the ones
            # matmul + PSUM copy of earlier revisions
            lsum = small.tile([1, 1], F32, tag="lsum")
            nc.gpsimd.tensor_reduce(
                lsum, ltail, axis=mybir.AxisListType.C, op=mybir.AluOpType.add
            )
            nc.sync.dma_start(out=out[:, :], in_=lsum)

    return nc


_NC = None


def _canonical_inds():
    idx = np.arange(B)
    not_self = ~np.eye(B, dtype=bool)
    neg1 = np.broadcast_to(idx[None, :], (B, B))[not_self].reshape(B, B - 1)
    neg2 = neg1 + B
    pos = (idx + B)[:, None]
    return np.concatenate([pos, neg1, neg2], axis=1)


_CANON = None


def _is_canonical(neigh_inds):
    global _CANON
    if neigh_inds.shape != (B, 2 * B - 1):
        return False
    if _CANON is None:
        _CANON = _canonical_inds()
    return np.array_equal(np.asarray(neigh_inds, dtype=np.int64), _CANON)


def _run_fast(feats):
    global _NC, LAST_RESULT

    if _NC is None:
        _NC = _build_canonical()
    fb = feats.astype(mybir.dt.np(BF16))
    in_maps = []
    for c in range(NCORES):
        order = [c, NCORES + c] + [
            blk for blk in range(16) if blk not in (c, NCORES + c)
        ]
        rows = np.concatenate([np.arange(blk * 128, (blk + 1) * 128) for blk in order])
        ftp = np.ascontiguousarray(fb[rows].T)
        in_maps.append({"ftp": ftp})
    res = run_bass_kernel_spmd(_NC, in_maps, list(range(NCORES)), trace=TRACE)
    LAST_RESULT = res
    total = sum(float(res.results[c]["out"][0, 0]) for c in range(NCORES))
    return np.asarray(total / B, dtype=np.float32)


def _run_general(feats, neigh_inds):
    """Correctness fallback for non-canonical neighbor indices."""
    b = feats.shape[0] // 2
    origs = feats[:b]
    gram = origs @ feats.T
    sq = np.sum(feats * feats, axis=1)
    dists = sq[:b, None] + sq[None, :] - 2.0 * gram
    probs = 1.0 / (1.0 + dists)
    rows = np.arange(b)[:, None]
    sel = probs[rows, np.asarray(neigh_inds, dtype=np.int64)]
    loss = -(np.log(sel[:, 0]) - np.log(np.sum(sel, axis=1)))
    return np.asarray(np.mean(loss), dtype=np.float32)


def kernel(features, neigh_inds):
    feats = np.ascontiguousarray(np.asarray(features, dtype=np.float32))
    ni = np.asarray(neigh_inds)
    if _is_canonical(ni):
        return _run_fast(feats)
    return _run_general(feats, ni)
